# revision 18
# baseline (speedup 1.0000x reference)
import numpy as np
import ml_dtypes

B, T, C = 4, 2048, 1024
H_PER_CORE = 8
HL = 512
D = 64
QC = 512
NQC = T // QC
NKC = T // 128
N_CORES = 8

_CACHE = {}


def _emit(nc, tc, tile, mybir, io):
    import concourse.bass as bass
    f32, bf16 = mybir.dt.float32, mybir.dt.bfloat16
    Exp = mybir.ActivationFunctionType.Exp
    xT, wq, wk, wv, wc, tril, out = (
        io["xT"], io["wq"], io["wk"], io["wv"], io["wc"],
        io["tril"], io["out"],
    )

    from contextlib import ExitStack

    with ExitStack() as ctx:
        persist = ctx.enter_context(tc.tile_pool(name="persist", bufs=1))
        qt = persist.tile([128, 4, T], bf16)
        kt = persist.tile([128, 4, T], bf16)
        ot = persist.tile([128, 4, T], bf16)
        vp = persist.tile([128, NKC, H_PER_CORE, D + 1], bf16)
        wq_sb = persist.tile([128, 8, HL], bf16, tag="wqs")
        wk_sb = persist.tile([128, 8, HL], bf16, tag="wks")
        wv_sb = persist.tile([128, 8, HL], bf16, tag="wvs")
        wc_sb = persist.tile([128, 4, C], bf16, tag="wcs")
        tril_sb = persist.tile([128, 2, 128], bf16, tag="tril")

        for kc in range(8):
            nc.gpsimd.dma_start(
                out=wq_sb[:, kc, :], in_=wq[kc * 128:(kc + 1) * 128, :])
            nc.gpsimd.dma_start(
                out=wk_sb[:, kc, :], in_=wk[kc * 128:(kc + 1) * 128, :])
        for kc in range(8):
            nc.gpsimd.dma_start(
                out=wv_sb[:, kc, :], in_=wv[kc * 128:(kc + 1) * 128, :])
        nc.gpsimd.dma_start(
            out=wc_sb, in_=wc.rearrange("(kd p) m -> p kd m", p=128))
        nc.vector.memset(vp[:, :, :, D], 1.0)
        nc.sync.dma_start(out=tril_sb[:, 0, :], in_=tril)
        nc.sync.dma_start(out=tril_sb[:, 1, :], in_=tril)

        pA = ctx.enter_context(tc.tile_pool(name="pA", bufs=2, space="PSUM"))
        pwp = ctx.enter_context(tc.tile_pool(name="pw", bufs=2, space="PSUM"))
        pop = ctx.enter_context(tc.tile_pool(name="po", bufs=2, space="PSUM"))
        xtp = ctx.enter_context(tc.tile_pool(name="xtp", bufs=16))
        ewp = ctx.enter_context(tc.tile_pool(name="ewp", bufs=6))
        dbp = ctx.enter_context(tc.tile_pool(name="dbp", bufs=2))
        stp = ctx.enter_context(tc.tile_pool(name="stp", bufs=4))
        drp = ctx.enter_context(tc.tile_pool(name="drp", bufs=4, space="DRAM"))

        x_tiles = {}

        def emit_x_dma(n):
            ts = []
            for kc in range(8):
                t = xtp.tile([128, QC], bf16, tag="xt")
                eng = nc.scalar if (n <= 1 and kc % 2 == 1) else nc.sync
                eng.dma_start(
                    out=t[:],
                    in_=xT[kc * 128:(kc + 1) * 128, n * QC:(n + 1) * QC])
                ts.append(t)
            x_tiles[n] = ts

        def xs(n, kc):
            return x_tiles[n][kc][:]

        def qk_group(n, mc, wsb, dst, ev):
            p = pA.tile([128, QC], f32, tag="pA")
            for kc in range(8):
                nc.tensor.matmul(
                    out=p[:], lhsT=wsb[:, kc, mc * 128:(mc + 1) * 128],
                    rhs=xs(n, kc), start=(kc == 0), stop=(kc == 7))
            dst_ap = dst[:, mc, n * QC:(n + 1) * QC]
            if ev == "act":
                nc.scalar.copy(dst_ap, p[:])
            else:
                nc.vector.tensor_copy(dst_ap, p[:])

        def v_group(n, mt, ev):
            p = pA.tile([128, QC], f32, tag="pA")
            for kc in range(8):
                nc.tensor.matmul(
                    out=p[:], lhsT=xs(n, kc)[:, mt * 128:(mt + 1) * 128],
                    rhs=wv_sb[:, kc, :], start=(kc == 0), stop=(kc == 7))
            gm = n * 4 + mt
            out_ap = vp[:, gm, :, 0:D]
            in_ap = p.rearrange("p (h d) -> p h d", d=D)
            if ev == "act":
                nc.scalar.copy(out_ap, in_ap)
            else:
                nc.vector.tensor_copy(out_ap, in_ap)

        def p3_group(qc, j, n2, ev, rotate=False):
            mt = 4 * qc + j
            if rotate:
                idx = (2 * j + n2) % 3
                if idx == 0:
                    p = pA.tile([128, QC], f32, tag="pA")
                elif idx == 1:
                    p = pop.tile([128, QC], f32, tag="po")
                else:
                    pwt = pwp.tile([128, 2, QC], f32, tag="pw")
                    p = pwt[:, 0, :]
            else:
                p = pA.tile([128, QC], f32, tag="pA")
            for kd in range(4):
                nc.tensor.matmul(
                    out=p[:],
                    lhsT=ot[:, kd, mt * 128:(mt + 1) * 128],
                    rhs=wc_sb[:, kd, n2 * QC:(n2 + 1) * QC],
                    start=(kd == 0), stop=(kd == 3))
            st = stp.tile([128, QC], bf16, tag="st")
            if ev == "act":
                nc.scalar.copy(st[:], p[:])
            else:
                nc.vector.tensor_copy(st[:], p[:])
            nc.sync.dma_start(
                out=out[mt * 128:(mt + 1) * 128, n2 * QC:(n2 + 1) * QC],
                in_=st[:])

        def so_block(qc):
            K = 4 * qc + 4
            LAG = 3
            pending_norm = [None]
            for hp in range(4):
                if pending_norm[0] is not None:
                    pending_norm[0]()
                    pending_norm[0] = None
                ha, hb = 2 * hp, 2 * hp + 1
                po_a = pop.tile([128, QC], f32, tag="po")
                po_b = pop.tile([128, QC], f32, tag="po")
                ews = {}

                def o_pair(kc, qc=qc, hp=hp, po_a=po_a, po_b=po_b, ews=ews):
                    off = (kc - 4 * qc) * 128 if kc >= 4 * qc else 0
                    ew = ews.pop(kc)
                    for hi, (h, po_t) in ((0, (ha, po_a)), (1, (hb, po_b))):
                        nc.tensor.matmul(
                            out=po_t[0:D + 1, off:QC],
                            lhsT=vp[:, kc, h, :],
                            rhs=ew[:, hi, off:QC],
                            start=(kc == 0), stop=(kc == K - 1),
                            skip_group_check=True)

                for kc in range(K):
                    off = (kc - 4 * qc) * 128 if kc >= 4 * qc else 0
                    pw_t = pwp.tile([128, 2, QC], f32, tag="pw")
                    for hi, r0 in ((0, 0), (1, 64)):
                        nc.tensor.matmul(
                            out=pw_t[:, hi, off:QC],
                            lhsT=kt[r0:r0 + 64, hp,
                                    kc * 128:(kc + 1) * 128],
                            rhs=qt[r0:r0 + 64, hp,
                                   qc * QC + off:(qc + 1) * QC],
                            start=True, stop=True, tile_position=(r0, 0))
                    ew = ewp.tile([128, 2, QC], bf16, tag="ew")
                    ews[kc] = ew
                    nc.scalar.activation(
                        ew[:, :, off:QC], pw_t[:, :, off:QC], Exp,
                        scale=0.125)
                    if kc >= 4 * qc:
                        nc.vector.tensor_mul(
                            ew[:, :, off:off + 128],
                            ew[:, :, off:off + 128],
                            tril_sb[:, :, :])
                    if kc >= LAG:
                        o_pair(kc - LAG)
                    yield
                for kc in range(max(0, K - LAG), K):
                    o_pair(kc)
                import contextlib
                hot = (qc == 3 and hp == 3)
                prio = tc.high_priority() if hot else contextlib.nullcontext()
                qsl = slice(qc * QC, (qc + 1) * QC)
                with prio:
                    nc.vector.tensor_copy(ot[0:64, hp, qsl], po_a[0:D, :])
                    nc.vector.tensor_copy(ot[64:128, hp, qsl], po_b[0:D, :])
                    d_sb = dbp.tile([1, 2, QC], f32, tag="dsb")
                    nc.vector.tensor_copy(d_sb[0:1, 0, :], po_a[D:D + 1, :])
                    nc.vector.tensor_copy(d_sb[0:1, 1, :], po_b[D:D + 1, :])
                    nc.vector.reciprocal_approx_fast(d_sb[:], d_sb[:])
                    dr = drp.tile([2, QC], f32, tag="dr")
                    (nc.scalar if hot else nc.sync).dma_start(
                        out=dr[:], in_=d_sb[:])
                    db = dbp.tile([128, QC], bf16, tag="db")
                    d0 = dr[:]
                    nc.gpsimd.dma_start(
                        out=db[:],
                        in_=bass.AP(tensor=d0.tensor, offset=d0.offset,
                                    ap=[[QC, 2], [0, 64], [1, QC]]))
                if hot:
                    with tc.high_priority():
                        nc.vector.tensor_mul(ot[:, hp, qsl], ot[:, hp, qsl],
                                             db[:])
                else:
                    pending_norm[0] = (
                        lambda hp=hp, qsl=qsl, db=db: nc.gpsimd.tensor_mul(
                            ot[:, hp, qsl], ot[:, hp, qsl], db[:]))
            if pending_norm[0] is not None:
                pending_norm[0]()

        def block_fillers(n):
            fs = []
            ev = "dve"
            if n + 1 < NQC:
                fs.append(lambda n=n: emit_x_dma(n + 1))
            if n < NQC:
                for mc in range(4):
                    fs.append(
                        lambda n=n, mc=mc: qk_group(n, mc, wq_sb, qt, ev))
                    fs.append(
                        lambda n=n, mc=mc: qk_group(n, mc, wk_sb, kt, ev))
                for mt in range(4):
                    fs.append(lambda n=n, mt=mt: v_group(n, mt, ev))
            if n >= 4:
                rot = (n == 5)
                for qc in (2 * (n - 4), 2 * (n - 4) + 1):
                    for j in range(4):
                        for n2 in range(2):
                            pev = "dve" if (n == 4 or (j + n2) % 2) else "act"
                            fs.append(lambda qc=qc, j=j, n2=n2, pev=pev,
                                      rot=rot:
                                      p3_group(qc, j, n2, pev, rot))
            return fs

        emit_x_dma(0)
        for n in range(6):
            fillers = block_fillers(n)
            if n == 0 or n == 5:
                for f in fillers:
                    f()
                continue
            qc = n - 1
            n_bi = 4 * (4 * qc + 4)
            rate = len(fillers) / n_bi
            acc = 0.0
            for _ in so_block(qc):
                acc += rate
                while acc >= 1.0 and fillers:
                    fillers.pop(0)()
                    acc -= 1.0
            for f in fillers:
                f()


def build_program():
    if "nc" in _CACHE:
        return _CACHE["nc"]
    import concourse.bacc as bacc
    import concourse.tile as tile
    from concourse import mybir

    f32, bf16 = mybir.dt.float32, mybir.dt.bfloat16
    nc = bacc.Bacc("TRN2", target_bir_lowering=False, debug=False,
                   num_devices=N_CORES)
    io = {
        "xT": nc.dram_tensor("xT", [C, T], bf16, kind="ExternalInput").ap(),
        "wq": nc.dram_tensor("wq", [C, HL], bf16, kind="ExternalInput").ap(),
        "wk": nc.dram_tensor("wk", [C, HL], bf16, kind="ExternalInput").ap(),
        "wv": nc.dram_tensor("wv", [C, HL], bf16, kind="ExternalInput").ap(),
        "wc": nc.dram_tensor("wc", [HL, C], bf16, kind="ExternalInput").ap(),
        "tril": nc.dram_tensor("tril", [128, 128], bf16,
                               kind="ExternalInput").ap(),
        "out": nc.dram_tensor("out", [T, C], bf16, kind="ExternalOutput").ap(),
    }
    with tile.TileContext(nc) as tc:
        _emit(nc, tc, tile, mybir, io)
    nc.compile()
    _CACHE["nc"] = nc
    return nc


def make_in_maps(x, Wq, Wk, Wv, Wc):
    bf16 = ml_dtypes.bfloat16
    x = np.asarray(x, dtype=np.float32)
    Wq = np.asarray(Wq, dtype=np.float32).astype(bf16)
    Wk = np.asarray(Wk, dtype=np.float32).astype(bf16)
    Wv = np.asarray(Wv, dtype=np.float32).astype(bf16)
    Wc = np.asarray(Wc, dtype=np.float32).astype(bf16)

    i_idx = np.arange(128)[:, None]
    j_idx = np.arange(128)[None, :]
    tril = (j_idx >= i_idx).astype(bf16)

    in_maps = []
    for b in range(B):
        xT = np.ascontiguousarray(x[b].T).astype(bf16)
        for g in range(2):
            sl = slice(g * HL, (g + 1) * HL)
            in_maps.append({
                "xT": xT,
                "wq": np.ascontiguousarray(Wq[:, sl]),
                "wk": np.ascontiguousarray(Wk[:, sl]),
                "wv": np.ascontiguousarray(Wv[:, sl]),
                "wc": np.ascontiguousarray(Wc[sl, :]),
                "tril": tril,
            })
    return in_maps


def kernel(x, Wq, Wk, Wv, Wc, bc):
    from concourse.bass_utils import run_bass_kernel_spmd

    nc = build_program()
    in_maps = make_in_maps(x, Wq, Wk, Wv, Wc)
    res = run_bass_kernel_spmd(nc, in_maps, core_ids=list(range(N_CORES)))
    bc = np.asarray(bc, dtype=np.float32)
    out = np.empty((B, T, C), dtype=np.float32)
    for b in range(B):
        out[b] = (res.results[2 * b]["out"].astype(np.float32)
                  + res.results[2 * b + 1]["out"].astype(np.float32) + bc)
    return out


# revision 19
# speedup vs baseline: 1.0778x; 1.0778x over previous
import numpy as np
import ml_dtypes

B, T, C = 4, 2048, 1024
H_PER_CORE = 8
HL = 512
D = 64
QC = 512
NQC = T // QC
NKC = T // 128
N_CORES = 8

_CACHE = {}


def _emit(nc, tc, tile, mybir, io):
    import concourse.bass as bass
    f32, bf16 = mybir.dt.float32, mybir.dt.bfloat16
    Exp = mybir.ActivationFunctionType.Exp
    xT, wq, wk, wv, wc, tril, out = (
        io["xT"], io["wq"], io["wk"], io["wv"], io["wc"],
        io["tril"], io["out"],
    )

    from contextlib import ExitStack

    with ExitStack() as ctx:
        persist = ctx.enter_context(tc.tile_pool(name="persist", bufs=1))
        qt = persist.tile([128, 4, T], bf16)
        kt = persist.tile([128, 4, T], bf16)
        ot = persist.tile([128, 4, T], bf16)
        vp = persist.tile([128, NKC, H_PER_CORE, D + 1], bf16)
        wq_sb = persist.tile([128, 8, HL], bf16, tag="wqs")
        wk_sb = persist.tile([128, 8, HL], bf16, tag="wks")
        wv_sb = persist.tile([128, 8, HL], bf16, tag="wvs")
        wc_sb = persist.tile([128, 4, C], bf16, tag="wcs")
        tril_sb = persist.tile([128, 2, 128], bf16, tag="tril")

        for kc in range(8):
            nc.gpsimd.dma_start(
                out=wq_sb[:, kc, :], in_=wq[kc * 128:(kc + 1) * 128, :])
            nc.gpsimd.dma_start(
                out=wk_sb[:, kc, :], in_=wk[kc * 128:(kc + 1) * 128, :])
        for kc in range(8):
            nc.gpsimd.dma_start(
                out=wv_sb[:, kc, :], in_=wv[kc * 128:(kc + 1) * 128, :])
        nc.gpsimd.dma_start(
            out=wc_sb, in_=wc.rearrange("(kd p) m -> p kd m", p=128))
        nc.vector.memset(vp[:, :, :, D], 1.0)
        nc.sync.dma_start(out=tril_sb[:, 0, :], in_=tril)
        nc.sync.dma_start(out=tril_sb[:, 1, :], in_=tril)

        pA = ctx.enter_context(tc.tile_pool(name="pA", bufs=2, space="PSUM"))
        pwp = ctx.enter_context(tc.tile_pool(name="pw", bufs=2, space="PSUM"))
        pop = ctx.enter_context(tc.tile_pool(name="po", bufs=2, space="PSUM"))
        xtp = ctx.enter_context(tc.tile_pool(name="xtp", bufs=16))
        ewp = ctx.enter_context(tc.tile_pool(name="ewp", bufs=6))
        dbp = ctx.enter_context(tc.tile_pool(name="dbp", bufs=2))
        stp = ctx.enter_context(tc.tile_pool(name="stp", bufs=4))
        drp = ctx.enter_context(tc.tile_pool(name="drp", bufs=4, space="DRAM"))

        x_tiles = {}

        def emit_x_dma(n):
            ts = []
            for kc in range(8):
                t = xtp.tile([128, QC], bf16, tag="xt")
                eng = nc.scalar if (n <= 1 and kc % 2 == 1) else nc.sync
                eng.dma_start(
                    out=t[:],
                    in_=xT[kc * 128:(kc + 1) * 128, n * QC:(n + 1) * QC])
                ts.append(t)
            x_tiles[n] = ts

        def xs(n, kc):
            return x_tiles[n][kc][:]

        def qk_group(n, mc, wsb, dst, ev):
            p = pA.tile([128, QC], f32, tag="pA")
            for kc in range(8):
                nc.tensor.matmul(
                    out=p[:], lhsT=wsb[:, kc, mc * 128:(mc + 1) * 128],
                    rhs=xs(n, kc), start=(kc == 0), stop=(kc == 7))
            dst_ap = dst[:, mc, n * QC:(n + 1) * QC]
            if ev == "act":
                nc.scalar.copy(dst_ap, p[:])
            else:
                nc.vector.tensor_copy(dst_ap, p[:])

        def v_group(n, mt, ev):
            p = pA.tile([128, QC], f32, tag="pA")
            for kc in range(8):
                nc.tensor.matmul(
                    out=p[:], lhsT=xs(n, kc)[:, mt * 128:(mt + 1) * 128],
                    rhs=wv_sb[:, kc, :], start=(kc == 0), stop=(kc == 7))
            gm = n * 4 + mt
            out_ap = vp[:, gm, :, 0:D]
            in_ap = p.rearrange("p (h d) -> p h d", d=D)
            if ev == "act":
                nc.scalar.copy(out_ap, in_ap)
            else:
                nc.vector.tensor_copy(out_ap, in_ap)

        def p3_group(qc, j, n2, ev):
            mt = 4 * qc + j
            p = pA.tile([128, QC], f32, tag="pA")
            for kd in range(4):
                nc.tensor.matmul(
                    out=p[:],
                    lhsT=ot[:, kd, mt * 128:(mt + 1) * 128],
                    rhs=wc_sb[:, kd, n2 * QC:(n2 + 1) * QC],
                    start=(kd == 0), stop=(kd == 3))
            st = stp.tile([128, QC], bf16, tag="st")
            if ev == "act":
                nc.scalar.copy(st[:], p[:])
            else:
                nc.vector.tensor_copy(st[:], p[:])
            nc.sync.dma_start(
                out=out[mt * 128:(mt + 1) * 128, n2 * QC:(n2 + 1) * QC],
                in_=st[:])

        def so_block(qc):
            K = 4 * qc + 4
            LAG = 3
            pending_norm = [None]
            for hp in range(4):
                if pending_norm[0] is not None:
                    pending_norm[0]()
                    pending_norm[0] = None
                ha, hb = 2 * hp, 2 * hp + 1
                po_a = pop.tile([128, QC], f32, tag="po")
                po_b = pop.tile([128, QC], f32, tag="po")
                ews = {}

                def o_pair(kc, qc=qc, hp=hp, po_a=po_a, po_b=po_b, ews=ews):
                    off = (kc - 4 * qc) * 128 if kc >= 4 * qc else 0
                    ew = ews.pop(kc)
                    for hi, (h, po_t) in ((0, (ha, po_a)), (1, (hb, po_b))):
                        nc.tensor.matmul(
                            out=po_t[0:D + 1, off:QC],
                            lhsT=vp[:, kc, h, :],
                            rhs=ew[:, hi, off:QC],
                            start=(kc == 0), stop=(kc == K - 1),
                            skip_group_check=True)

                for kc in range(K):
                    off = (kc - 4 * qc) * 128 if kc >= 4 * qc else 0
                    pw_t = pwp.tile([128, 2, QC], f32, tag="pw")
                    for hi, r0 in ((0, 0), (1, 64)):
                        nc.tensor.matmul(
                            out=pw_t[:, hi, off:QC],
                            lhsT=kt[r0:r0 + 64, hp,
                                    kc * 128:(kc + 1) * 128],
                            rhs=qt[r0:r0 + 64, hp,
                                   qc * QC + off:(qc + 1) * QC],
                            start=True, stop=True, tile_position=(r0, 0))
                    ew = ewp.tile([128, 2, QC], bf16, tag="ew")
                    ews[kc] = ew
                    nc.scalar.activation(
                        ew[:, :, off:QC], pw_t[:, :, off:QC], Exp,
                        scale=0.125)
                    if kc >= 4 * qc:
                        nc.vector.tensor_mul(
                            ew[:, :, off:off + 128],
                            ew[:, :, off:off + 128],
                            tril_sb[:, :, :])
                    if kc >= LAG:
                        o_pair(kc - LAG)
                    yield
                for kc in range(max(0, K - LAG), K):
                    o_pair(kc)
                import contextlib
                hot = (qc == 3 and hp == 3)
                prio = tc.high_priority() if hot else contextlib.nullcontext()
                qsl = slice(qc * QC, (qc + 1) * QC)
                with prio:
                    nc.vector.tensor_copy(ot[0:64, hp, qsl], po_a[0:D, :])
                    nc.vector.tensor_copy(ot[64:128, hp, qsl], po_b[0:D, :])
                    d_sb = dbp.tile([1, 2, QC], f32, tag="dsb")
                    nc.vector.tensor_copy(d_sb[0:1, 0, :], po_a[D:D + 1, :])
                    nc.vector.tensor_copy(d_sb[0:1, 1, :], po_b[D:D + 1, :])
                    nc.vector.reciprocal_approx_fast(d_sb[:], d_sb[:])
                    dr = drp.tile([2, QC], f32, tag="dr")
                    (nc.scalar if hot else nc.sync).dma_start(
                        out=dr[:], in_=d_sb[:])
                    db = dbp.tile([128, QC], bf16, tag="db")
                    d0 = dr[:]
                    nc.gpsimd.dma_start(
                        out=db[:],
                        in_=bass.AP(tensor=d0.tensor, offset=d0.offset,
                                    ap=[[QC, 2], [0, 64], [1, QC]]))
                if hot:
                    nc.gpsimd.tensor_mul(ot[:, hp, qsl], ot[:, hp, qsl],
                                         db[:])
                else:
                    pending_norm[0] = (
                        lambda hp=hp, qsl=qsl, db=db: nc.gpsimd.tensor_mul(
                            ot[:, hp, qsl], ot[:, hp, qsl], db[:]))
            if pending_norm[0] is not None:
                pending_norm[0]()

        def block_fillers(n):
            fs = []
            ev = "dve"
            if n + 1 < NQC:
                fs.append(lambda n=n: emit_x_dma(n + 1))
            if n < NQC:
                for mc in range(4):
                    fs.append(
                        lambda n=n, mc=mc: qk_group(n, mc, wq_sb, qt, ev))
                    fs.append(
                        lambda n=n, mc=mc: qk_group(n, mc, wk_sb, kt, ev))
                for mt in range(4):
                    fs.append(lambda n=n, mt=mt: v_group(n, mt, ev))
            if n >= 4:
                for qc in (2 * (n - 4), 2 * (n - 4) + 1):
                    for j in range(4):
                        for n2 in range(2):
                            pev = "dve" if (n == 4 or (j + n2) % 2) else "act"
                            fs.append(lambda qc=qc, j=j, n2=n2, pev=pev:
                                      p3_group(qc, j, n2, pev))
            return fs

        emit_x_dma(0)
        for n in range(6):
            fillers = block_fillers(n)
            if n == 0 or n == 5:
                for f in fillers:
                    f()
                continue
            qc = n - 1
            n_bi = 4 * (4 * qc + 4)
            rate = len(fillers) / n_bi
            acc = 0.0
            for _ in so_block(qc):
                acc += rate
                while acc >= 1.0 and fillers:
                    fillers.pop(0)()
                    acc -= 1.0
            for f in fillers:
                f()


def build_program():
    if "nc" in _CACHE:
        return _CACHE["nc"]
    import concourse.bacc as bacc
    import concourse.tile as tile
    from concourse import mybir

    f32, bf16 = mybir.dt.float32, mybir.dt.bfloat16
    nc = bacc.Bacc("TRN2", target_bir_lowering=False, debug=False,
                   num_devices=N_CORES)
    io = {
        "xT": nc.dram_tensor("xT", [C, T], bf16, kind="ExternalInput").ap(),
        "wq": nc.dram_tensor("wq", [C, HL], bf16, kind="ExternalInput").ap(),
        "wk": nc.dram_tensor("wk", [C, HL], bf16, kind="ExternalInput").ap(),
        "wv": nc.dram_tensor("wv", [C, HL], bf16, kind="ExternalInput").ap(),
        "wc": nc.dram_tensor("wc", [HL, C], bf16, kind="ExternalInput").ap(),
        "tril": nc.dram_tensor("tril", [128, 128], bf16,
                               kind="ExternalInput").ap(),
        "out": nc.dram_tensor("out", [T, C], bf16, kind="ExternalOutput").ap(),
    }
    with tile.TileContext(nc) as tc:
        _emit(nc, tc, tile, mybir, io)
    nc.compile()
    _CACHE["nc"] = nc
    return nc


def make_in_maps(x, Wq, Wk, Wv, Wc):
    bf16 = ml_dtypes.bfloat16
    x = np.asarray(x, dtype=np.float32)
    Wq = np.asarray(Wq, dtype=np.float32).astype(bf16)
    Wk = np.asarray(Wk, dtype=np.float32).astype(bf16)
    Wv = np.asarray(Wv, dtype=np.float32).astype(bf16)
    Wc = np.asarray(Wc, dtype=np.float32).astype(bf16)

    i_idx = np.arange(128)[:, None]
    j_idx = np.arange(128)[None, :]
    tril = (j_idx >= i_idx).astype(bf16)

    in_maps = []
    for b in range(B):
        xT = np.ascontiguousarray(x[b].T).astype(bf16)
        for g in range(2):
            sl = slice(g * HL, (g + 1) * HL)
            in_maps.append({
                "xT": xT,
                "wq": np.ascontiguousarray(Wq[:, sl]),
                "wk": np.ascontiguousarray(Wk[:, sl]),
                "wv": np.ascontiguousarray(Wv[:, sl]),
                "wc": np.ascontiguousarray(Wc[sl, :]),
                "tril": tril,
            })
    return in_maps


def kernel(x, Wq, Wk, Wv, Wc, bc):
    from concourse.bass_utils import run_bass_kernel_spmd

    nc = build_program()
    in_maps = make_in_maps(x, Wq, Wk, Wv, Wc)
    res = run_bass_kernel_spmd(nc, in_maps, core_ids=list(range(N_CORES)))
    bc = np.asarray(bc, dtype=np.float32)
    out = np.empty((B, T, C), dtype=np.float32)
    for b in range(B):
        out[b] = (res.results[2 * b]["out"].astype(np.float32)
                  + res.results[2 * b + 1]["out"].astype(np.float32) + bc)
    return out


# revision 21
# speedup vs baseline: 1.0785x; 1.0006x over previous
import numpy as np
import ml_dtypes

B, T, C = 4, 2048, 1024
H_PER_CORE = 8
HL = 512
D = 64
QC = 512
NQC = T // QC
NKC = T // 128
N_CORES = 8

_CACHE = {}


def _emit(nc, tc, tile, mybir, io):
    import contextlib
    import concourse.bass as bass
    f32, bf16 = mybir.dt.float32, mybir.dt.bfloat16
    Exp = mybir.ActivationFunctionType.Exp
    xT, wq, wk, wv, wc, tril, out = (
        io["xT"], io["wq"], io["wk"], io["wv"], io["wc"],
        io["tril"], io["out"],
    )

    from contextlib import ExitStack

    with ExitStack() as ctx:
        persist = ctx.enter_context(tc.tile_pool(name="persist", bufs=1))
        qt = persist.tile([128, 4, T], bf16)
        kt = persist.tile([128, 4, T], bf16)
        ot = persist.tile([128, 4, T], bf16)
        vp = persist.tile([128, NKC, H_PER_CORE, D + 1], bf16)
        wq_sb = persist.tile([128, 8, HL], bf16, tag="wqs")
        wk_sb = persist.tile([128, 8, HL], bf16, tag="wks")
        wv_sb = persist.tile([128, 8, HL], bf16, tag="wvs")
        wc_sb = persist.tile([128, 4, C], bf16, tag="wcs")
        tril_sb = persist.tile([128, 2, 128], bf16, tag="tril")

        for kc in range(8):
            nc.gpsimd.dma_start(
                out=wq_sb[:, kc, :], in_=wq[kc * 128:(kc + 1) * 128, :])
            nc.gpsimd.dma_start(
                out=wk_sb[:, kc, :], in_=wk[kc * 128:(kc + 1) * 128, :])
        for kc in range(8):
            nc.gpsimd.dma_start(
                out=wv_sb[:, kc, :], in_=wv[kc * 128:(kc + 1) * 128, :])
        nc.gpsimd.dma_start(
            out=wc_sb, in_=wc.rearrange("(kd p) m -> p kd m", p=128))
        nc.vector.memset(vp[:, :, :, D], 1.0)
        nc.sync.dma_start(out=tril_sb[:, 0, :], in_=tril)
        nc.sync.dma_start(out=tril_sb[:, 1, :], in_=tril)

        pA = ctx.enter_context(tc.tile_pool(name="pA", bufs=2, space="PSUM"))
        pwp = ctx.enter_context(tc.tile_pool(name="pw", bufs=2, space="PSUM"))
        pop = ctx.enter_context(tc.tile_pool(name="po", bufs=2, space="PSUM"))
        xtp = ctx.enter_context(tc.tile_pool(name="xtp", bufs=16))
        ewp = ctx.enter_context(tc.tile_pool(name="ewp", bufs=6))
        dbp = ctx.enter_context(tc.tile_pool(name="dbp", bufs=2))
        stp = ctx.enter_context(tc.tile_pool(name="stp", bufs=4))
        drp = ctx.enter_context(tc.tile_pool(name="drp", bufs=4, space="DRAM"))

        x_tiles = {}

        def emit_x_dma(n):
            ts = []
            for kc in range(8):
                t = xtp.tile([128, QC], bf16, tag="xt")
                eng = nc.scalar if (n <= 1 and kc % 2 == 1) else nc.sync
                eng.dma_start(
                    out=t[:],
                    in_=xT[kc * 128:(kc + 1) * 128, n * QC:(n + 1) * QC])
                ts.append(t)
            x_tiles[n] = ts

        def xs(n, kc):
            return x_tiles[n][kc][:]

        def qk_group(n, mc, wsb, dst, ev):
            p = pA.tile([128, QC], f32, tag="pA")
            for kc in range(8):
                nc.tensor.matmul(
                    out=p[:], lhsT=wsb[:, kc, mc * 128:(mc + 1) * 128],
                    rhs=xs(n, kc), start=(kc == 0), stop=(kc == 7))
            dst_ap = dst[:, mc, n * QC:(n + 1) * QC]
            if ev == "act":
                nc.scalar.copy(dst_ap, p[:])
            else:
                nc.vector.tensor_copy(dst_ap, p[:])

        def v_group(n, mt, ev):
            p = pA.tile([128, QC], f32, tag="pA")
            for kc in range(8):
                nc.tensor.matmul(
                    out=p[:], lhsT=xs(n, kc)[:, mt * 128:(mt + 1) * 128],
                    rhs=wv_sb[:, kc, :], start=(kc == 0), stop=(kc == 7))
            gm = n * 4 + mt
            out_ap = vp[:, gm, :, 0:D]
            in_ap = p.rearrange("p (h d) -> p h d", d=D)
            if ev == "act":
                nc.scalar.copy(out_ap, in_ap)
            else:
                nc.vector.tensor_copy(out_ap, in_ap)

        def p3_group(qc, j, n2, ev):
            mt = 4 * qc + j
            p = pA.tile([128, QC], f32, tag="pA")
            for kd in range(4):
                nc.tensor.matmul(
                    out=p[:],
                    lhsT=ot[:, kd, mt * 128:(mt + 1) * 128],
                    rhs=wc_sb[:, kd, n2 * QC:(n2 + 1) * QC],
                    start=(kd == 0), stop=(kd == 3))
            st = stp.tile([128, QC], bf16, tag="st")
            if ev == "act":
                nc.scalar.copy(st[:], p[:])
            else:
                nc.vector.tensor_copy(st[:], p[:])
            (nc.scalar if ev == "act" else nc.sync).dma_start(
                out=out[mt * 128:(mt + 1) * 128, n2 * QC:(n2 + 1) * QC],
                in_=st[:])

        def so_block(qc):
            K = 4 * qc + 4
            LAG = 3
            pending_norm = [None]
            for hp in range(4):
                if pending_norm[0] is not None:
                    pending_norm[0]()
                    pending_norm[0] = None
                po_a = pop.tile([128, QC], f32, tag="po")
                po_b = pop.tile([128, QC], f32, tag="po")
                ews = {}

                def o_pair(kc, qc=qc, hp=hp, po_a=po_a, po_b=po_b, ews=ews):
                    off = (kc - 4 * qc) * 128 if kc >= 4 * qc else 0
                    ew = ews.pop(kc)
                    for hi, (h, po_t) in ((0, (2 * hp, po_a)),
                                          (1, (2 * hp + 1, po_b))):
                        nc.tensor.matmul(
                            out=po_t[0:D + 1, off:QC],
                            lhsT=vp[:, kc, h, :],
                            rhs=ew[:, hi, off:QC],
                            start=(kc == 0), stop=(kc == K - 1),
                            skip_group_check=True)

                for kc in range(K):
                    off = (kc - 4 * qc) * 128 if kc >= 4 * qc else 0
                    pw_t = pwp.tile([128, 2, QC], f32, tag="pw")
                    for hi, r0 in ((0, 0), (1, 64)):
                        nc.tensor.matmul(
                            out=pw_t[:, hi, off:QC],
                            lhsT=kt[r0:r0 + 64, hp,
                                    kc * 128:(kc + 1) * 128],
                            rhs=qt[r0:r0 + 64, hp,
                                   qc * QC + off:(qc + 1) * QC],
                            start=True, stop=True, tile_position=(r0, 0))
                    ew = ewp.tile([128, 2, QC], bf16, tag="ew")
                    ews[kc] = ew
                    nc.scalar.activation(
                        ew[:, :, off:QC], pw_t[:, :, off:QC], Exp,
                        scale=0.125)
                    if kc >= 4 * qc:
                        nc.vector.tensor_mul(
                            ew[:, :, off:off + 128],
                            ew[:, :, off:off + 128],
                            tril_sb[:, :, :])
                    if kc >= LAG:
                        o_pair(kc - LAG)
                    yield
                for kc in range(max(0, K - LAG), K):
                    o_pair(kc)
                hot = (qc == 3 and hp == 3)
                prio = tc.high_priority() if hot else contextlib.nullcontext()
                qsl = slice(qc * QC, (qc + 1) * QC)
                with prio:
                    d_sb = dbp.tile([1, 2, QC], f32, tag="dsb")
                    nc.vector.tensor_copy(d_sb[0:1, 0, :], po_a[D:D + 1, :])
                    nc.vector.tensor_copy(d_sb[0:1, 1, :], po_b[D:D + 1, :])
                    nc.vector.reciprocal_approx_fast(d_sb[:], d_sb[:])
                    dr = drp.tile([2, QC], f32, tag="dr")
                    (nc.scalar if hot else nc.sync).dma_start(
                        out=dr[:], in_=d_sb[:])
                    db = dbp.tile([128, QC], bf16, tag="db")
                    d0 = dr[:]
                    nc.gpsimd.dma_start(
                        out=db[:],
                        in_=bass.AP(tensor=d0.tensor, offset=d0.offset,
                                    ap=[[QC, 2], [0, 64], [1, QC]]))
                    nc.vector.tensor_copy(ot[0:64, hp, qsl], po_a[0:D, :])
                    nc.vector.tensor_copy(ot[64:128, hp, qsl], po_b[0:D, :])
                if qc == 3:
                    nc.gpsimd.tensor_mul(ot[:, hp, qsl], ot[:, hp, qsl],
                                         db[:])
                else:
                    pending_norm[0] = (
                        lambda hp=hp, qsl=qsl, db=db: nc.gpsimd.tensor_mul(
                            ot[:, hp, qsl], ot[:, hp, qsl], db[:]))
            if pending_norm[0] is not None:
                pending_norm[0]()

        def block_fillers(n):
            fs = []
            ev = "dve"
            if n + 1 < NQC:
                fs.append(lambda n=n: emit_x_dma(n + 1))
            if n < NQC:
                for mc in range(4):
                    fs.append(
                        lambda n=n, mc=mc: qk_group(n, mc, wq_sb, qt, ev))
                    fs.append(
                        lambda n=n, mc=mc: qk_group(n, mc, wk_sb, kt, ev))
                for mt in range(4):
                    fs.append(lambda n=n, mt=mt: v_group(n, mt, ev))
            if n >= 4:
                for qc in (2 * (n - 4), 2 * (n - 4) + 1):
                    for j in range(4):
                        for n2 in range(2):
                            pev = "dve" if (n == 4 or (j + n2) % 2) else "act"
                            fs.append(lambda qc=qc, j=j, n2=n2, pev=pev:
                                      p3_group(qc, j, n2, pev))
            return fs

        emit_x_dma(0)
        for n in range(6):
            fillers = block_fillers(n)
            if n == 0 or n == 5:
                for f in fillers:
                    f()
                continue
            qc = n - 1
            n_bi = 4 * (4 * qc + 4)
            rate = len(fillers) / n_bi
            acc = 0.0
            for _ in so_block(qc):
                acc += rate
                while acc >= 1.0 and fillers:
                    fillers.pop(0)()
                    acc -= 1.0
            for f in fillers:
                f()


def build_program():
    if "nc" in _CACHE:
        return _CACHE["nc"]
    import concourse.bacc as bacc
    import concourse.tile as tile
    from concourse import mybir

    bf16 = mybir.dt.bfloat16
    nc = bacc.Bacc("TRN2", target_bir_lowering=False, debug=False,
                   num_devices=N_CORES)
    io = {
        "xT": nc.dram_tensor("xT", [C, T], bf16, kind="ExternalInput").ap(),
        "wq": nc.dram_tensor("wq", [C, HL], bf16, kind="ExternalInput").ap(),
        "wk": nc.dram_tensor("wk", [C, HL], bf16, kind="ExternalInput").ap(),
        "wv": nc.dram_tensor("wv", [C, HL], bf16, kind="ExternalInput").ap(),
        "wc": nc.dram_tensor("wc", [HL, C], bf16, kind="ExternalInput").ap(),
        "tril": nc.dram_tensor("tril", [128, 128], bf16,
                               kind="ExternalInput").ap(),
        "out": nc.dram_tensor("out", [T, C], bf16, kind="ExternalOutput").ap(),
    }
    with tile.TileContext(nc) as tc:
        _emit(nc, tc, tile, mybir, io)
    nc.compile()
    _CACHE["nc"] = nc
    return nc


def make_in_maps(x, Wq, Wk, Wv, Wc):
    bf16 = ml_dtypes.bfloat16
    x = np.asarray(x, dtype=np.float32)
    Wq = np.asarray(Wq, dtype=np.float32).astype(bf16)
    Wk = np.asarray(Wk, dtype=np.float32).astype(bf16)
    Wv = np.asarray(Wv, dtype=np.float32).astype(bf16)
    Wc = np.asarray(Wc, dtype=np.float32).astype(bf16)

    i_idx = np.arange(128)[:, None]
    j_idx = np.arange(128)[None, :]
    tril = (j_idx >= i_idx).astype(bf16)

    in_maps = []
    for b in range(B):
        xT = np.ascontiguousarray(x[b].T).astype(bf16)
        for g in range(2):
            sl = slice(g * HL, (g + 1) * HL)
            in_maps.append({
                "xT": xT,
                "wq": np.ascontiguousarray(Wq[:, sl]),
                "wk": np.ascontiguousarray(Wk[:, sl]),
                "wv": np.ascontiguousarray(Wv[:, sl]),
                "wc": np.ascontiguousarray(Wc[sl, :]),
                "tril": tril,
            })
    return in_maps


def kernel(x, Wq, Wk, Wv, Wc, bc):
    from concourse.bass_utils import run_bass_kernel_spmd

    nc = build_program()
    in_maps = make_in_maps(x, Wq, Wk, Wv, Wc)
    res = run_bass_kernel_spmd(nc, in_maps, core_ids=list(range(N_CORES)))
    bc = np.asarray(bc, dtype=np.float32)
    out = np.empty((B, T, C), dtype=np.float32)
    for b in range(B):
        out[b] = (res.results[2 * b]["out"].astype(np.float32)
                  + res.results[2 * b + 1]["out"].astype(np.float32) + bc)
    return out
